# revision 35
# baseline (speedup 1.0000x reference)
"""Trainium2 Bass kernel for BeamSearchOptim (beam search decode).

Strategy: vocab-shard w_out across the 8 cores (each core holds a
[1024, 4000] float32r slice resident in SBUF).  Every step each core:
  - gathers emb rows for the current tokens (dma_gather, 64 beam rows,
    row order r = b + 16*j so the [16,4] token tile is the index layout)
  - PE-transposes them to h^T layout and adds enc^T (output float32r)
  - computes its logits slice [64, 4000] in four [64, 1000] PSUM pieces
    using float32r matmuls (single-pass fp32 on the PE, 4x fp32 rate)
  - per piece: ACT copies logits to SBUF + computes sum(exp(logit - C)),
    DVE max/max_index extracts the piece top-8 (top-4 used) so only the
    last piece's top-k sits on the serial tail
  - packs [vals 4x8 | token-ids 4x4 | sumexp] = [64, 49] and AllGathers
  - every core redundantly merges: per batch row, top-4 of 512
    candidates (4 beams x 8 cores x 4 pieces x 4), updates
    scores/finished, records token+parent history
Final seqs/lengths/length-penalty/argmax are reconstructed on the host
from the token/parent history (exact, tiny).
"""

import sys

sys.path.insert(0, "/opt/trn_rl_repo")

import numpy as np

from concourse import bass, mybir, bacc, tile  # noqa: E402
from concourse import bass_utils  # noqa: E402

# ---- problem config (hardcoded, mirrors the reference) ----
BATCH = 16
B_BEAM = 4
BB = BATCH * B_BEAM  # 64
MAX_LEN = 64
SOS, EOS = 1, 2
ALPHA = 0.6
VOCAB, D = 32000, 1024
NEG_INF = -1e30
N_CORES = 8
VSH = VOCAB // N_CORES  # 4000 vocab columns per core
NPIECE = 4
PW = VSH // NPIECE  # 1000 columns per piece
CBIAS = 16.0  # fixed logsumexp shift (replaces row max)
P = 128
NCAND = 512  # merge candidates per batch row: 4j x 8c x 4p x 4s

f32 = mybir.dt.float32
f32r = mybir.dt.float32r
i16 = mybir.dt.int16
u8 = mybir.dt.uint8
u32 = mybir.dt.uint32
AF = mybir.ActivationFunctionType
OP = mybir.AluOpType
AX = mybir.AxisListType


def _ap_append0(a, n):
    """Append a stride-0 (broadcast) innermost dim of size n to an AP."""
    return bass.AP(a.tensor, a.offset, list(a.ap) + [[0, n]])


def build_nc(n_steps=MAX_LEN, enable_asserts=False):
    nc = bacc.Bacc(
        "TRN2",
        target_bir_lowering=False,
        debug=False,
        enable_asserts=enable_asserts,
        num_devices=N_CORES,
    )

    # --- kernel I/O ---
    wsl = nc.dram_tensor("wsl", [D, VSH], f32, kind="ExternalInput").ap()
    emb = nc.dram_tensor("emb", [VOCAB, D], f32, kind="ExternalInput").ap()
    encT = nc.dram_tensor("encT", [P, 512], f32, kind="ExternalInput").ap()
    id64 = nc.dram_tensor("id64", [64, 64], f32, kind="ExternalInput").ap()
    selm = nc.dram_tensor("selm", [64, 64], f32, kind="ExternalInput").ap()
    tokoff = nc.dram_tensor("tokoff", [P, 1], f32, kind="ExternalInput").ap()
    iota1k = nc.dram_tensor("iota1k", [16, 1024], f32, kind="ExternalInput").ap()
    iota16 = nc.dram_tensor("iota16", [16, 16], f32, kind="ExternalInput").ap()
    tmpl = nc.dram_tensor("tmpl", [16, NCAND], f32, kind="ExternalInput").ap()
    initi = nc.dram_tensor("initi", [P, 4], i16, kind="ExternalInput").ap()
    repm = nc.dram_tensor("repm", [16, 128], f32, kind="ExternalInput").ap()

    th_o = nc.dram_tensor("th_o", [16, 4 * n_steps], f32, kind="ExternalOutput").ap()
    ph_o = nc.dram_tensor("ph_o", [16, 4 * n_steps], f32, kind="ExternalOutput").ap()
    sc_o = nc.dram_tensor("sc_o", [16, 4], f32, kind="ExternalOutput").ap()

    with tile.TileContext(nc) as tc:
        with (
            tc.tile_pool(name="wpool", bufs=1) as wpool,
            tc.tile_pool(name="cpool", bufs=1) as cpool,
            tc.tile_pool(name="hpool", bufs=1) as hpool,
            tc.tile_pool(name="gpool", bufs=1) as gpool,
            tc.tile_pool(name="htpool", bufs=2) as htpool,
            tc.tile_pool(name="lgpool", bufs=1) as lgpool,
            tc.tile_pool(name="smpool", bufs=2) as smpool,
            tc.tile_pool(name="stpool", bufs=2) as stpool,
            tc.tile_pool(name="psmm", bufs=2, space="PSUM") as psmm,
            tc.tile_pool(name="pstp", bufs=1, space="PSUM") as pstp,
            tc.tile_pool(name="psmg", bufs=1, space="PSUM") as psmg,
            tc.tile_pool(name="dpool", bufs=2, space="DRAM") as dpool,
            tc.tile_pool(name="dshpool", bufs=2, space="DRAM") as dshpool,
        ):
            # --- resident weights: 8 K-chunks of the vocab slice (f32r) ---
            w_sb = []
            for k in range(8):
                wk = wpool.tile([P, VSH], f32r, name=f"w{k}", tag=f"w{k}")
                nc.gpsimd.dma_start(wk[:], wsl[128 * k : 128 * (k + 1), :])
                w_sb.append(wk)

            # --- constants ---
            encT_sb = cpool.tile([P, 512], f32, name="encT_sb")
            nc.sync.dma_start(encT_sb[:], encT)
            id64_sb = cpool.tile([64, 64], f32, name="id64_sb")
            nc.sync.dma_start(id64_sb[:], id64)
            sel_sb = cpool.tile([64, 64], f32, name="sel_sb")
            nc.sync.dma_start(sel_sb[:], selm)
            tokoff_sb = cpool.tile([P, 1], f32, name="tokoff_sb")
            nc.sync.dma_start(tokoff_sb[:], tokoff)
            iota1k_sb = cpool.tile([16, 1024], f32, name="iota1k_sb")
            nc.sync.dma_start(iota1k_sb[:], iota1k)
            iota16_sb = cpool.tile([16, 16], f32, name="iota16_sb")
            nc.sync.dma_start(iota16_sb[:], iota16)
            tmpl_sb = cpool.tile([16, NCAND], f32, name="tmpl_sb")
            nc.sync.dma_start(tmpl_sb[:], tmpl)
            cbias_sb = cpool.tile([P, 1], f32, name="cbias_sb")
            nc.vector.memset(cbias_sb[:], -CBIAS)
            repm_sb = cpool.tile([16, 128], f32, name="repm_sb")
            nc.sync.dma_start(repm_sb[:], repm)

            # --- persistent state ---
            th_sb = hpool.tile([16, 4 * n_steps], f32, name="th_sb")
            ph_sb = hpool.tile([16, 4 * n_steps], f32, name="ph_sb")
            scores_sb = stpool.tile([16, 4], f32, name="scores0", tag="scores")
            nc.vector.memset(scores_sb[:], 0.0)
            fin_sb = stpool.tile([16, 4], f32, name="fin0", tag="fin")
            nc.vector.memset(fin_sb[:], 0.0)
            finb_cur = stpool.tile([16, NCAND], u8, name="finb0", tag="finb")
            nc.vector.memset(finb_cur[:], 0)
            candF_cur = stpool.tile([16, NCAND], f32, name="candF0", tag="candF")
            nc.vector.tensor_copy(candF_cur[:], tmpl_sb[:])

            def do_gather(t):
                """Gather emb rows for step t's tokens (row r <- emb[tok[r]]).
                Token indices [128, 4] int16: row r = b + 16j, wrapped in 16
                partitions, replicated x8 for dma_gather."""
                idx16 = smpool.tile([P, 4], i16, name=f"idx16_{t}", tag="idx16")
                if t == 0:
                    nc.sync.dma_start(idx16[:], initi)
                else:
                    idxp = pstp.tile([P, 4], f32, name=f"idxp_{t}", tag="tp")
                    nc.tensor.matmul(
                        idxp[:], repm_sb[:], th_sb[:, 4 * (t - 1) : 4 * t],
                        start=True, stop=True, skip_group_check=True,
                    )
                    nc.vector.tensor_copy(idx16[:], idxp[:])
                gg = gpool.tile([P, D], f32, name=f"gg_{t}", tag="gg")
                nc.gpsimd.dma_gather(
                    out_ap=gg[:].rearrange("p (a m) -> p a m", a=1),
                    in_ap=emb,
                    idxs_ap=idx16[:],
                    num_idxs=64,
                    num_idxs_reg=64,
                    elem_size=D,
                )
                return gg

            gg_cur = do_gather(0)

            for t in range(n_steps):
                gg = gg_cur
                # ---- transpose to h^T chunks, add enc^T ----
                tp = pstp.tile([P, 512], f32, name=f"tp_{t}", tag="tp")
                for k in range(8):
                    nc.tensor.transpose(
                        tp[:, 64 * k : 64 * (k + 1)],
                        gg[0:64, 128 * k : 128 * (k + 1)],
                        id64_sb[:],
                    )
                hT = htpool.tile([P, 512], f32r, name=f"hT_{t}", tag="hT")
                nc.vector.tensor_add(hT[:], tp[:], encT_sb[:])

                # ---- vocab-slice matmuls in 4 pieces + per-piece post ----
                logits = lgpool.tile([64, VSH], f32, name=f"logits_{t}", tag="logits")
                Ssum = smpool.tile([64, NPIECE], f32, name=f"Ssum_{t}", tag="Ssum")
                gl = smpool.tile([64, 49], f32, name=f"gl_{t}", tag="gl")
                for p_ in range(NPIECE):
                    ps = psmm.tile([64, 1024], f32, name=f"ps_{t}_{p_}", tag="ps")
                    for k in range(8):
                        lhsT = hT[:, 64 * k : 64 * (k + 1)]
                        st = k == 0
                        sp = k == 7
                        for (o_, w_) in ((0, 512), (512, 488)):
                            col = PW * p_ + o_
                            nc.tensor.matmul(
                                ps[0:64, o_ : o_ + w_],
                                lhsT,
                                w_sb[k][:, col : col + w_],
                                start=st,
                                stop=sp,
                                skip_group_check=True,
                            )
                    lg_sl = logits[:, PW * p_ : PW * (p_ + 1)]
                    nc.scalar.activation(lg_sl, ps[0:64, 0:PW], AF.Copy)
                    expb = gpool.tile([64, PW], f32, name=f"expb_{t}_{p_}", tag="expb")
                    nc.scalar.activation(
                        expb[:],
                        ps[0:64, 0:PW],
                        AF.Exp,
                        bias=cbias_sb[0:64, 0:1],
                        scale=1.0,
                        accum_out=Ssum[:, p_ : p_ + 1],
                    )
                    # per-piece local top-8 + global token ids
                    nc.vector.max(out=gl[:, 8 * p_ : 8 * (p_ + 1)], in_=lg_sl)
                    idx8 = smpool.tile([64, 8], u32, name=f"idx8_{t}_{p_}", tag="idx8")
                    nc.vector.max_index(
                        out=idx8[:], in_max=gl[:, 8 * p_ : 8 * (p_ + 1)], in_values=lg_sl
                    )
                    nc.vector.tensor_scalar(
                        gl[:, 32 + 4 * p_ : 36 + 4 * p_],
                        idx8[:, 0:4],
                        tokoff_sb[0:64, 0:1],
                        float(PW * p_),
                        op0=OP.add,
                        op1=OP.add,
                    )
                nc.vector.tensor_reduce(
                    gl[:, 48:49], Ssum[:], axis=AX.X, op=OP.add
                )

                # ---- allgather ----
                cc_in = dpool.tile([64, 49], f32, name=f"cc_in_{t}", tag="cc_in")
                cc_out = dshpool.tile(
                    [64 * 8, 49], f32, name=f"cc_out_{t}", tag="cc_out",
                    addr_space="Shared",
                )
                nc.sync.dma_start(cc_in[:], gl[:])
                nc.gpsimd.collective_compute(
                    "AllGather",
                    OP.bypass,
                    replica_groups=[list(range(N_CORES))],
                    ins=[cc_in[:].opt()],
                    outs=[cc_out[:].opt()],
                )
                cc_sb = gpool.tile([64, 392], f32, name=f"cc_sb_{t}", tag="cc_sb")
                # single strided load: partition r <- rows {r, 64+r, ...}
                nc.sync.dma_start(
                    cc_sb[:].rearrange("p (c f) -> p c f", f=49),
                    cc_out.rearrange("(c p) f -> p c f", c=8),
                )

                # ---- assemble merge views via selection matmuls ----
                # cand slot: 128*j + 16*c + 4*p + s
                mv = psmg.tile([16, NCAND], f32, name=f"mv_{t}", tag="mv")
                mt = psmg.tile([16, NCAND], f32, name=f"mt_{t}", tag="mt")
                msum = psmg.tile([16, 32], f32, name=f"msum_{t}", tag="msum")
                cc3 = cc_sb[:].rearrange("p (c f) -> p c f", f=49)
                rhs_v = cc3[:, :, 0:32].rearrange("p c (x s) -> p c x s", s=8)[
                    :, :, :, 0:4
                ]
                rhs_t = cc3[:, :, 32:48].rearrange("p c (x s) -> p c x s", s=4)
                rhs_s = cc3[:, :, 48:49]
                # S first so the lse chain (sg -> Ln -> adjU) starts early
                for j in range(4):
                    nc.tensor.matmul(
                        msum[:, 8 * j : 8 * (j + 1)], sel_sb[:, 16 * j : 16 * (j + 1)],
                        rhs_s, start=True, stop=True, skip_group_check=True,
                    )
                for j in range(4):
                    selg = sel_sb[:, 16 * j : 16 * (j + 1)]
                    nc.tensor.matmul(
                        mv[:, 128 * j : 128 * (j + 1)], selg, rhs_v,
                        start=True, stop=True, skip_group_check=True,
                    )
                    nc.tensor.matmul(
                        mt[:, 128 * j : 128 * (j + 1)], selg, rhs_t,
                        start=True, stop=True, skip_group_check=True,
                    )

                # ---- merge (replicated on every core) ----
                sg = smpool.tile([16, 4], f32, name=f"sg_{t}", tag="sg")
                nc.vector.tensor_reduce(
                    sg[:],
                    msum[:].rearrange("p (j x) -> p j x", x=8),
                    axis=AX.X,
                    op=OP.add,
                )
                logS = smpool.tile([16, 4], f32, name=f"logS_{t}", tag="logS")
                nc.scalar.activation(logS[:], sg[:], AF.Ln)
                adjU = smpool.tile([16, 4], f32, name=f"adjU_{t}", tag="adjU")
                nc.vector.tensor_scalar(
                    adjU[:], logS[:], -1.0, -CBIAS, op0=OP.mult, op1=OP.add
                )
                nc.vector.tensor_add(adjU[:], adjU[:], scores_sb[:])

                cand = smpool.tile([16, NCAND], f32, name=f"cand_{t}", tag="cand")
                nc.vector.tensor_tensor(
                    cand[:].rearrange("p (j x) -> p j x", x=128),
                    mv[:].rearrange("p (j x) -> p j x", x=128),
                    _ap_append0(adjU[:], 128),
                    op=OP.add,
                )
                nc.vector.copy_predicated(cand[:], finb_cur[:], candF_cur[:])

                win8 = smpool.tile([16, 8], f32, name=f"win8_{t}", tag="win8")
                nc.vector.max(out=win8[:], in_=cand[:])
                wini = smpool.tile([16, 8], u32, name=f"wini_{t}", tag="wini")
                nc.vector.max_index(out=wini[:], in_max=win8[:], in_values=cand[:])
                winf = smpool.tile([16, 4], f32, name=f"winf_{t}", tag="winf")
                nc.vector.tensor_copy(winf[:], wini[:, 0:4])

                # tokens first: the gather chain (idxp -> idx16 -> dma_gather)
                # is the critical path; all other bookkeeping comes after
                ohd = smpool.tile([16, NCAND], f32, name=f"ohd_{t}", tag="ohd")
                for w in range(4):
                    nc.vector.scalar_tensor_tensor(
                        out=ohd[:],
                        in0=iota1k_sb[:, 0:NCAND],
                        scalar=winf[:, w : w + 1],
                        in1=mt[:],
                        op0=OP.is_equal,
                        op1=OP.mult,
                        accum_out=th_sb[:, 4 * t + w : 4 * t + w + 1],
                    )

                # next step's gather fires as soon as tokens exist
                if t + 1 < n_steps:
                    gg_cur = do_gather(t + 1)

                # parents = (slot >= 128) + (slot >= 256) + (slot >= 384)
                pges = smpool.tile([16, 12], f32, name=f"pges_{t}", tag="pges")
                nc.vector.tensor_scalar(pges[:, 0:4], winf[:], 128.0, None, op0=OP.is_ge)
                nc.vector.tensor_scalar(pges[:, 4:8], winf[:], 256.0, None, op0=OP.is_ge)
                nc.vector.tensor_scalar(pges[:, 8:12], winf[:], 384.0, None, op0=OP.is_ge)
                nc.vector.tensor_add(pges[:, 0:4], pges[:, 0:4], pges[:, 4:8])
                nc.vector.tensor_add(
                    ph_sb[:, 4 * t : 4 * (t + 1)], pges[:, 0:4], pges[:, 8:12]
                )

                # finished update
                te = smpool.tile([16, 4], f32, name=f"te_{t}", tag="te")
                nc.vector.tensor_scalar(
                    te[:], th_sb[:, 4 * t : 4 * (t + 1)], float(EOS), None,
                    op0=OP.is_equal,
                )
                ohj = smpool.tile([16, 16], f32, name=f"ohj_{t}", tag="ohj")
                nc.vector.tensor_tensor(
                    ohj[:].rearrange("p (w x) -> p w x", x=4),
                    iota16_sb[:].rearrange("p (w x) -> p w x", x=4),
                    _ap_append0(ph_sb[:, 4 * t : 4 * (t + 1)], 4),
                    op=OP.is_equal,
                )
                nc.vector.tensor_tensor(
                    ohj[:].rearrange("p (w x) -> p w x", x=4),
                    ohj[:].rearrange("p (w x) -> p w x", x=4),
                    bass.AP(fin_sb[:].tensor, fin_sb[:].offset,
                            [list(fin_sb[:].ap)[0], [0, 4]] + list(fin_sb[:].ap)[1:]),
                    op=OP.mult,
                )
                fin_new = stpool.tile([16, 4], f32, name=f"fin_{t}", tag="fin")
                nc.vector.tensor_reduce(
                    fin_new[:],
                    ohj[:].rearrange("p (w x) -> p w x", x=4),
                    axis=AX.X,
                    op=OP.max,
                )
                nc.vector.tensor_tensor(fin_new[:], fin_new[:], te[:], op=OP.max)
                sc_new = stpool.tile([16, 4], f32, name=f"sc_{t}", tag="scores")
                nc.vector.tensor_copy(sc_new[:], win8[:, 0:4])
                scores_sb, fin_sb = sc_new, fin_new
                # prepare next step's finished mask + finished-candidate
                # template off the critical chain
                if t + 1 < n_steps:
                    finb_cur = stpool.tile(
                        [16, NCAND], u8, name=f"finb_{t}", tag="finb"
                    )
                    nc.vector.tensor_copy(
                        finb_cur[:].rearrange("p (j x) -> p j x", x=128),
                        _ap_append0(fin_sb[:], 128),
                    )
                    candF_cur = stpool.tile(
                        [16, NCAND], f32, name=f"candF_{t}", tag="candF"
                    )
                    nc.vector.tensor_tensor(
                        candF_cur[:].rearrange("p (j x) -> p j x", x=128),
                        tmpl_sb[:].rearrange("p (j x) -> p j x", x=128),
                        _ap_append0(scores_sb[:], 128),
                        op=OP.add,
                    )

            # ---- outputs ----
            nc.sync.dma_start(th_o, th_sb[:])
            nc.sync.dma_start(ph_o, ph_sb[:])
            nc.sync.dma_start(sc_o, scores_sb[:])

    nc.compile()
    return nc


def make_inmaps(enc, emb, w_out):
    enc = np.asarray(enc, dtype=np.float32)
    emb = np.asarray(emb, dtype=np.float32)
    w_out = np.asarray(w_out, dtype=np.float32)
    # beam row order: r = b + 16*j
    enc_rep = np.tile(enc, (B_BEAM, 1))  # [64, 1024], row r -> batch r % 16
    encT = np.ascontiguousarray(
        enc_rep.reshape(64, 8, 128).transpose(2, 1, 0).reshape(128, 512)
    )
    id64 = np.eye(64, dtype=np.float32)
    sel = np.zeros((64, 64), np.float32)
    for j in range(4):
        for b in range(16):
            sel[b + 16 * j, 16 * j + b] = 1.0
    iota1k = np.tile(np.arange(1024, dtype=np.float32), (16, 1))
    iota16 = np.tile(np.arange(4, dtype=np.float32), (16, 4))
    tmpl = np.full((16, NCAND), NEG_INF, np.float32)
    tmpl[:, 128 * np.arange(4)] = 0.0
    initi = np.full((128, 4), SOS, np.int16)
    repm = np.tile(np.eye(16, dtype=np.float32), (1, 8))  # [16, 128]

    in_maps = []
    for c in range(N_CORES):
        tokoff = np.full((128, 1), c * VSH, np.float32)
        in_maps.append(
            {
                "wsl": np.ascontiguousarray(w_out[:, c * VSH : (c + 1) * VSH]),
                "emb": emb,
                "encT": encT,
                "id64": id64,
                "selm": sel,
                "tokoff": tokoff,
                "iota1k": iota1k,
                "iota16": iota16,
                "tmpl": tmpl,
                "initi": initi,
                "repm": repm,
            }
        )
    return in_maps


def postprocess(th, ph, sc, n_steps):
    """Reconstruct (seqs, final_scores) from token/parent history."""
    th = np.asarray(th, dtype=np.float64).reshape(16, n_steps, 4)
    ph = np.asarray(ph, dtype=np.float64).reshape(16, n_steps, 4).astype(np.int64)
    sc = np.asarray(sc, dtype=np.float64)  # [16, 4]
    seqs = np.full((BATCH, n_steps + 1), SOS, np.int32)
    final_scores = np.zeros(BATCH, np.float32)
    for b in range(BATCH):
        toks = np.zeros((n_steps, 4), np.int64)
        cur = np.arange(4)
        for t in range(n_steps - 1, -1, -1):
            toks[t, :] = th[b, t, cur].astype(np.int64)
            cur = ph[b, t, cur]
        lengths = np.empty(4, np.float64)
        for j in range(4):
            e = np.nonzero(toks[:, j] == EOS)[0]
            if len(e):
                t0 = int(e[0])
                lengths[j] = 1 + t0
                toks[t0:, j] = EOS
            else:
                lengths[j] = 1 + n_steps
        lpen = ((5.0 + lengths) / 6.0) ** ALPHA
        fs = (sc[b] / lpen).astype(np.float32)
        best = int(np.argmax(fs))
        seqs[b, 1 : n_steps + 1] = toks[:, best]
        final_scores[b] = fs[best]
    return seqs, final_scores


_NC_CACHE = {}


def _get_nc(n_steps):
    if n_steps not in _NC_CACHE:
        _NC_CACHE[n_steps] = build_nc(n_steps)
    return _NC_CACHE[n_steps]


def run_hw(enc, emb, w_out, n_steps=MAX_LEN, trace=False):
    nc = _get_nc(n_steps)
    in_maps = make_inmaps(enc, emb, w_out)
    res = bass_utils.run_bass_kernel_spmd(
        nc, in_maps, list(range(N_CORES)), trace=trace
    )
    r0 = res.results[0]
    out = postprocess(r0["th_o"], r0["ph_o"], r0["sc_o"], n_steps)
    return out, res


def run_sim(enc, emb, w_out, n_steps, num_workers=8):
    from concourse.bass_interp import MultiCoreSim

    nc = build_nc(n_steps)
    in_maps = make_inmaps(enc, emb, w_out)
    sim = MultiCoreSim(
        nc,
        num_cores=N_CORES,
        trace=False,
        require_finite=False,
        require_nnan=True,
        num_workers=num_workers,
    )
    for c, core in sim.cores.items():
        for name, arr in in_maps[c].items():
            core.tensor(name)[:] = arr
    sim.simulate(check_with_hw=False)
    print(f"[sim] modeled global_time: {sim.global_time} ns "
          f"({sim.global_time / max(n_steps,1):.0f} ns/step)")
    r0 = {n: np.array(sim.cores[0].tensor(n)) for n in ("th_o", "ph_o", "sc_o")}
    return postprocess(r0["th_o"], r0["ph_o"], r0["sc_o"], n_steps)


def _pjrt_callable(nc):
    """Build a repeatable jitted callable over device-resident inputs."""
    import jax
    from jax.sharding import Mesh, PartitionSpec, NamedSharding
    from jax.experimental.shard_map import shard_map
    from concourse import bass2jax, mybir as mb

    bass2jax.install_neuronx_cc_hook()
    partition_name = nc.partition_id_tensor.name if nc.partition_id_tensor else None
    in_names, out_names, out_avals, zero_outs = [], [], [], []
    for alloc in nc.m.functions[0].allocations:
        if not isinstance(alloc, mb.MemoryLocationSet):
            continue
        name = alloc.memorylocations[0].name
        if alloc.kind == "ExternalInput":
            if name != partition_name:
                in_names.append(name)
        elif alloc.kind == "ExternalOutput":
            out_names.append(name)
            shape = tuple(alloc.tensor_shape)
            dtype = mb.dt.np(alloc.dtype)
            out_avals.append(jax.core.ShapedArray(shape, dtype))
            zero_outs.append(np.zeros(shape, dtype))
    n_params = len(in_names)
    all_names = in_names + out_names
    if partition_name is not None:
        all_names = all_names + [partition_name]

    def _body(*args):
        operands = list(args)
        if partition_name is not None:
            operands.append(bass2jax.partition_id_tensor())
        outs = bass2jax._bass_exec_p.bind(
            *operands,
            out_avals=tuple(out_avals),
            in_names=tuple(all_names),
            out_names=tuple(out_names),
            lowering_input_output_aliases=(),
            sim_require_finite=True,
            sim_require_nnan=True,
            nc=nc,
        )
        return tuple(outs)

    devices = jax.devices()[:N_CORES]
    mesh = Mesh(np.asarray(devices), ("core",))
    n_outs = len(out_avals)
    sharded = jax.jit(
        shard_map(
            _body,
            mesh=mesh,
            in_specs=(PartitionSpec("core"),) * (n_params + n_outs),
            out_specs=(PartitionSpec("core"),) * n_outs,
            check_rep=False,
        ),
        donate_argnums=tuple(range(n_params, n_params + n_outs)),
        keep_unused=True,
    )
    sharding = NamedSharding(mesh, PartitionSpec("core"))
    return sharded, in_names, out_names, zero_outs, sharding, out_avals


def bench_hw(enc, emb, w_out, n_steps=MAX_LEN, iters=10):
    """Returns (per-iter wall times ns, outputs from last run)."""
    import time as _time
    import jax

    nc = _get_nc(n_steps)
    in_maps = make_inmaps(enc, emb, w_out)
    sharded, in_names, out_names, zero_outs, sharding, out_avals = _pjrt_callable(nc)
    concat_in = [
        np.concatenate([in_maps[c][n] for c in range(N_CORES)], axis=0)
        for n in in_names
    ]
    dev_in = [jax.device_put(a, sharding) for a in concat_in]
    times = []
    outs = None
    for _ in range(iters):
        dev_zeros = [
            jax.device_put(
                np.zeros((N_CORES * z.shape[0], *z.shape[1:]), z.dtype), sharding
            )
            for z in zero_outs
        ]
        jax.block_until_ready(dev_zeros)
        t0 = _time.perf_counter()
        outs = sharded(*dev_in, *dev_zeros)
        jax.block_until_ready(outs)
        times.append((_time.perf_counter() - t0) * 1e9)
    r0 = {
        n: np.asarray(outs[i]).reshape(N_CORES, *out_avals[i].shape)[0]
        for i, n in enumerate(out_names)
    }
    result = postprocess(r0["th_o"], r0["ph_o"], r0["sc_o"], n_steps)
    return times, result


def kernel(enc, emb, w_out):
    (out, _res) = run_hw(enc, emb, w_out, MAX_LEN)
    return out
